# revision 11
# baseline (speedup 1.0000x reference)
"""Trainium2 Bass kernel for nn_DotAttention_19765439497049.

reference math:
    nq  = l2norm(query, -1)              # [B, Q, D]
    nk  = l2norm(key, -1)                # [B, W, S, D]
    sim = einsum('bqd,bwsd->bqws', nq, nk)
    sim = l2norm(sim, -1)                # normalize over S
    out = einsum('bqws,bwsd->bqwd', sim, key)

Key identities used:
  * the query normalization cancels inside the S-axis l2norm (uniform
    positive scale per (b, q)), so the kernel never normalizes the query.
  * |out[q,w,d]| <= ||sim_w[q]/nrm|| * ||key[w,:,d]|| = ||key[w,:,d]||_2
    (Cauchy-Schwarz, the normalized sim is a unit vector).  The host
    folds 127/S0 (S0 = padded max column norm) into the raw key, so the
    device emits the output as int8 with no extra engine work; the host
    multiplies back by S0/127.  Quantization error ~0.4% of absmax,
    well inside the 2e-2 gate.

I/O per core: query bf16 (2.1 MB) in, key bf16 (tiny) in, out int8
(5.2 MB) -> ~7.9 MB of DMA vs 25.3 MB for the f32 version.

Per 128-query tile:
  PE  : mm1 sim = qT^T @ nkT (4 matmuls, 320 free) and, one tile
        delayed, mm2 out_w = simT_w^T @ key_w (5 matmuls, 512 free)
  DVE : sim PSUM->SBUF bf16 copy, sum-of-squares reduce, merged w0/w1
        scaled int8 copy (rinv broadcast via stride-0 AP)
  ACT : rsqrt, w2/w3/w4 scaled int8 copies (per-partition scale AP)
  Pool: square (SBUF only - no PSUM port), xbar pad memset
  DMA : q loads (4-tile groups), one xbar transpose [128,384]->[128,3,128]
        per tile, int8 stores (2-tile groups)

Sharding: 8 cores = 4 batches x 2 query-halves; outputs disjoint.
"""

import numpy as np

B, Q, W, S, D = 4, 4096, 5, 64, 512
NCORES = 8
QSH = Q * B // NCORES      # 2048 queries per core
PT = 128                   # queries per inner tile
NT = QSH // PT             # 16 inner tiles
WS = W * S                 # 320
WPAD = 384                 # 3 x 128, xbar needs free dim % 128 == 0
KC = D // 128              # 4 contraction chunks
LD_B = 4                   # q-tiles per input DMA
ST_B = 2                   # q-tiles per output DMA
SKEW = 2                   # tiles between mm1/xbar and mm2/copies (hides
                           # the xbar's DGE + transfer + sem latency)
MARGIN = 1.015             # int8 headroom over the exact C-S bound

_CACHE = {}


def _build():
    import concourse.bacc as bacc
    import concourse.tile as tile
    from concourse import mybir, masks

    f32 = mybir.dt.float32
    bf16 = mybir.dt.bfloat16
    i8 = mybir.dt.int8
    AF = mybir.ActivationFunctionType

    nc = bacc.Bacc("TRN2", target_bir_lowering=False, debug=False)
    # chunk-major host-transposed bf16 query: qT[c, dl, q] = query[q, c*128+dl]
    qT_d = nc.dram_tensor("qT", [KC, 128, QSH], bf16, kind="ExternalInput").ap()
    # chunk-major host-normalized+transposed key for mm1
    nkT_d = nc.dram_tensor("nkT", [KC, 128, WS], bf16, kind="ExternalInput").ap()
    # raw key scaled by 127/S0, laid out [s, w, d] for mm2's moving operand
    ks_d = nc.dram_tensor("ks", [S, W, D], bf16, kind="ExternalInput").ap()
    o_d = nc.dram_tensor("out", [QSH, W * D], i8, kind="ExternalOutput").ap()

    with tile.TileContext(nc) as tc:
        with (
            tc.tile_pool(name="const", bufs=1) as constp,
            tc.tile_pool(name="keyp", bufs=1) as keyp,
            tc.tile_pool(name="qin", bufs=2) as qin,
            tc.tile_pool(name="simcp", bufs=4) as simcp,
            tc.tile_pool(name="simtp", bufs=4) as simtp,
            tc.tile_pool(name="sqp", bufs=2) as sqp,
            tc.tile_pool(name="nrmp", bufs=4) as nrmp,
            tc.tile_pool(name="outp", bufs=3) as outp,
            tc.tile_pool(name="ps_sim", bufs=1, space="PSUM") as ps_sim,
            tc.tile_pool(name="ps_o01", bufs=2, space="PSUM") as ps_o01,
            tc.tile_pool(name="ps_o23", bufs=1, space="PSUM") as ps_o23,
            tc.tile_pool(name="ps_o4", bufs=1, space="PSUM") as ps_o4,
        ):
            ident = constp.tile([128, 128], f32)
            masks.make_identity(nc, ident[:])

            # warm the PE p-state while the first DMAs are in flight
            warm_ps = ps_sim.tile([PT, WS], f32, tag="sim_ps")
            for i in range(12):
                nc.tensor.transpose(warm_ps[:, (i % 2) * 128:(i % 2 + 1) * 128],
                                    ident[:], ident[:])
            # touch the ACT function tables off the critical path
            dummy = constp.tile([1, 8], f32)
            nc.scalar.sqrt(dummy[:, 0:2], ident[0:1, 0:2])
            nc.scalar.mul(dummy[:, 4:6], ident[0:1, 0:2], 1.0)

            # ---- key loads (small; gate mm1/mm2) ------------------------
            nkT = keyp.tile([128, KC * WS], bf16, tag="nkT")
            nc.sync.dma_start(
                nkT[:].rearrange("p (c n) -> p c n", n=WS),
                nkT_d[:, :, :].rearrange("c p n -> p c n"))
            # key replicated into both partition halves so mm2's rhs base
            # partition matches simT's (w%2)*64 slice
            key_sb = keyp.tile([128, W * D], bf16, tag="key_sb")
            for hh in range(2):
                nc.sync.dma_start(
                    key_sb[hh * S:(hh + 1) * S, :],
                    ks_d[:, :, :].rearrange("s w d -> s (w d)"))

            st = {}  # per-tile state for the 1-tile software pipeline

            def emit_back_half(u):
                simT, rinv = st.pop(u)
                # mm2 per w; w0/w1 share a 2-bank psum tile for the merged
                # DVE copy, w2/w3 share one for ACT's singles
                o01 = ps_o01.tile([PT, 1024], f32, tag="o01")
                o23 = ps_o23.tile([PT, 1024], f32, tag="o23")
                o4 = ps_o4.tile([PT, 512], f32, tag="o4")
                for w in range(W):
                    dst = (o01[:, (w % 2) * 512:(w % 2) * 512 + 512] if w < 2
                           else o23[:, (w % 2) * 512:(w % 2) * 512 + 512] if w < 4
                           else o4[:])
                    hh = w % 2
                    nc.tensor.matmul(
                        dst,
                        simT[hh * S:(hh + 1) * S, w // 2, :],
                        key_sb[hh * S:(hh + 1) * S, w * D:(w + 1) * D],
                        start=True, stop=True)

                if u % ST_B == 0:
                    st["ob"] = outp.tile([PT, ST_B * W * D], i8, tag="ob",
                                         name="ob")
                ob = st["ob"]
                ob_off = (u % ST_B) * W * D
                # w0/w1: one DVE tensor_tensor, rinv broadcast along d
                nc.vector.tensor_tensor(
                    ob[:, ob_off:ob_off + 1024].rearrange(
                        "p (w d) -> p w d", d=512),
                    o01[:].rearrange("p (w d) -> p w d", d=512),
                    rinv[:, 0:2, None].broadcast_to((PT, 2, 512)),
                    mybir.AluOpType.mult)
                # w2/w3/w4: ACT scaled copies (per-partition scale AP)
                nc.scalar.mul(ob[:, ob_off + 1024:ob_off + 1536],
                              o23[:, 0:512], rinv[:, 2:3])
                nc.scalar.mul(ob[:, ob_off + 1536:ob_off + 2048],
                              o23[:, 512:1024], rinv[:, 3:4])
                nc.scalar.mul(ob[:, ob_off + 2048:ob_off + 2560],
                              o4[:], rinv[:, 4:5])

                if u % ST_B == ST_B - 1:
                    g0 = u - ST_B + 1
                    nc.sync.dma_start(
                        o_d[g0 * PT:(g0 + ST_B) * PT, :]
                        .rearrange("(j p) d -> p j d", p=PT),
                        ob[:].rearrange("p (j d) -> p j d", d=W * D))

            # ---- main loop over query tiles -----------------------------
            qg = None
            for t in range(NT):
                if t % LD_B == 0:
                    qg = qin.tile([128, KC * LD_B * PT], bf16, tag="qg")
                    nc.sync.dma_start(
                        qg[:].rearrange("p (c q) -> p c q", q=LD_B * PT),
                        qT_d[:, :, t * PT:(t + LD_B) * PT]
                        .rearrange("c p q -> p c q"))
                qoff = (t % LD_B) * PT

                # mm1: sim[q, ws] = sum_D qT^T nkT
                sim_ps = ps_sim.tile([PT, WS], f32, tag="sim_ps")
                for c in range(KC):
                    nc.tensor.matmul(
                        sim_ps[:],
                        qg[:, c * LD_B * PT + qoff:c * LD_B * PT + qoff + PT],
                        nkT[:, c * WS:(c + 1) * WS],
                        start=(c == 0), stop=(c == KC - 1))

                # sim -> SBUF bf16 (feeds the xbar transpose + Pool square)
                simc = simcp.tile([PT, WPAD], bf16, tag="simc")
                nc.vector.tensor_copy(simc[:, 0:WS], sim_ps[:])
                if t < 4:  # zero each rotating buffer's xbar pad once
                    nc.gpsimd.memset(simc[:, WS:WPAD], 0)

                # norm path: ssq = sum_s sim^2, rinv = rsqrt(ssq)
                sq = sqp.tile([PT, WS], f32, tag="sq")
                nc.gpsimd.tensor_mul(sq[:], simc[:, 0:WS], simc[:, 0:WS])
                ssq = nrmp.tile([PT, 8], f32, tag="ssq")
                nc.vector.reduce_sum(
                    out=ssq[:, 0:W],
                    in_=sq[:].rearrange("p (w s) -> p w s", s=S),
                    axis=mybir.AxisListType.X)
                nrm = nrmp.tile([PT, 8], f32, tag="nrm")
                nc.scalar.sqrt(nrm[:, 0:W], ssq[:, 0:W])
                rinv = nrmp.tile([PT, 8], f32, tag="rinv")
                nc.vector.reciprocal(rinv[:, 0:W], nrm[:, 0:W])

                # xbar transpose: simT[s2w, wgrp, q] = simc[q, wgrp*128+s2w]
                simT = simtp.tile([128, 3, PT], bf16, tag="simT")
                nc.sync.dma_start_transpose(simT[:], simc[:])

                st[t] = (simT, rinv)
                if t >= SKEW:
                    emit_back_half(t - SKEW)
            for u in range(NT - SKEW, NT):
                emit_back_half(u)

    nc.compile()
    return nc


def _get_nc():
    if "nc" not in _CACHE:
        _CACHE["nc"] = _build()
    return _CACHE["nc"]


def kernel(query: np.ndarray, key: np.ndarray) -> np.ndarray:
    import ml_dtypes
    from concourse.bass_utils import run_bass_kernel_spmd

    bf16 = ml_dtypes.bfloat16
    query = np.asarray(query, dtype=np.float32)
    key = np.asarray(key, dtype=np.float32)
    assert query.shape == (B, Q, D) and key.shape == (B, W, S, D)

    nc = _get_nc()
    half = Q // 2
    in_maps = []
    scales = []
    for core in range(NCORES):
        b, h = divmod(core, 2)
        qs = query[b, h * half:(h + 1) * half, :]             # [QSH, D]
        kb = key[b].reshape(W * S, D).astype(np.float64)      # [WS, D]
        n = np.linalg.norm(kb, axis=-1, keepdims=True)
        nk = kb / np.maximum(n, 1e-12)
        # int8 bound: |out*rinv| <= max_col ||key[w,:,d]||_2
        s0 = MARGIN * np.sqrt((key[b].astype(np.float64) ** 2)
                              .sum(axis=1)).max()
        scales.append(np.float32(s0 / 127.0))
        ks = (key[b].transpose(1, 0, 2) * (127.0 / s0))       # [S, W, D]
        in_maps.append({
            "qT": np.ascontiguousarray(qs.T).reshape(KC, 128, QSH)
                    .astype(bf16),
            "nkT": np.ascontiguousarray(nk.T.astype(np.float32))
                    .reshape(KC, 128, WS).astype(bf16),
            "ks": np.ascontiguousarray(ks).astype(bf16),
        })
    res = run_bass_kernel_spmd(nc, in_maps, list(range(NCORES)))
    out = np.empty((B, Q, W, D), dtype=np.float32)
    for core in range(NCORES):
        b, h = divmod(core, 2)
        o8 = res.results[core]["out"].reshape(half, W, D)
        out[b, h * half:(h + 1) * half] = \
            o8.astype(np.float32) * scales[core]
    return out


# revision 15
# speedup vs baseline: 1.1386x; 1.1386x over previous
"""Trainium2 Bass kernel for nn_DotAttention_19765439497049.

reference math:
    nq  = l2norm(query, -1)              # [B, Q, D]
    nk  = l2norm(key, -1)                # [B, W, S, D]
    sim = einsum('bqd,bwsd->bqws', nq, nk)
    sim = l2norm(sim, -1)                # normalize over S
    out = einsum('bqws,bwsd->bqwd', sim, key)

Key identities used:
  * the query normalization cancels inside the S-axis l2norm (uniform
    positive scale per (b, q)), so the kernel never normalizes the query.
  * |out[q,w,d]| <= ||sim_w[q]/nrm|| * ||key[w,:,d]|| = ||key[w,:,d]||_2
    (Cauchy-Schwarz, the normalized sim is a unit vector).  The host
    folds 127/S0 (S0 = padded max column norm) into the raw key, so the
    device emits the output as int8 with no extra engine work; the host
    multiplies back by S0/127.  Quantization error ~0.4% of absmax,
    well inside the 2e-2 gate.

I/O per core: query bf16 (2.1 MB) in, key bf16 (tiny) in, out int8
(5.2 MB) -> ~7.9 MB of DMA vs 25.3 MB for the f32 version.

Per 128-query tile:
  PE  : mm1 sim = qT^T @ nkT (4 matmuls, 320 free) and, one tile
        delayed, mm2 out_w = simT_w^T @ key_w (5 matmuls, 512 free)
  DVE : sim PSUM->SBUF bf16 copy, sum-of-squares reduce, merged w0/w1
        scaled int8 copy (rinv broadcast via stride-0 AP)
  ACT : rsqrt, w2/w3/w4 scaled int8 copies (per-partition scale AP)
  Pool: square (SBUF only - no PSUM port), xbar pad memset
  DMA : q loads (4-tile groups), one xbar transpose [128,384]->[128,3,128]
        per tile, int8 stores (2-tile groups)

Sharding: 8 cores = 4 batches x 2 query-halves; outputs disjoint.
"""

import numpy as np

B, Q, W, S, D = 4, 4096, 5, 64, 512
NCORES = 8
QSH = Q * B // NCORES      # 2048 queries per core
PT = 128                   # queries per inner tile
NT = QSH // PT             # 16 inner tiles
WS = W * S                 # 320
WPAD = 384                 # 3 x 128, xbar needs free dim % 128 == 0
KC = D // 128              # 4 contraction chunks
QCHUNKS = [(0, 2), (2, 6), (6, 16)]  # q-tile prefetch chunks (all issued
                                     # up front; first is small for latency)
ST_B = 2                   # q-tiles per output DMA
SKEW = 2                   # tiles between mm1/xbar and mm2/copies (hides
                           # the xbar's DGE + transfer + sem latency)
MARGIN = 1.015             # int8 headroom over the exact C-S bound

_CACHE = {}


def _build():
    import concourse.bacc as bacc
    import concourse.tile as tile
    from concourse import mybir, masks

    f32 = mybir.dt.float32
    bf16 = mybir.dt.bfloat16
    i8 = mybir.dt.int8
    AF = mybir.ActivationFunctionType

    nc = bacc.Bacc("TRN2", target_bir_lowering=False, debug=False)
    # chunk-major host-transposed bf16 query: qT[c, dl, q] = query[q, c*128+dl]
    qT_d = nc.dram_tensor("qT", [KC, 128, QSH], bf16, kind="ExternalInput").ap()
    # chunk-major host-normalized+transposed key for mm1
    nkT_d = nc.dram_tensor("nkT", [KC, 128, WS], bf16, kind="ExternalInput").ap()
    # raw key scaled by 127/S0, laid out [s, w, d] for mm2's moving operand
    ks_d = nc.dram_tensor("ks", [S, W, D], bf16, kind="ExternalInput").ap()
    o_d = nc.dram_tensor("out", [QSH, W * D], i8, kind="ExternalOutput").ap()

    with tile.TileContext(nc) as tc:
        with (
            tc.tile_pool(name="const", bufs=1) as constp,
            tc.tile_pool(name="keyp", bufs=1) as keyp,
            tc.tile_pool(name="simcp", bufs=4) as simcp,
            tc.tile_pool(name="simtp", bufs=4) as simtp,
            tc.tile_pool(name="sqp", bufs=2) as sqp,
            tc.tile_pool(name="nrmp", bufs=4) as nrmp,
            tc.tile_pool(name="outp", bufs=3) as outp,
            tc.tile_pool(name="ps_sim", bufs=1, space="PSUM") as ps_sim,
            tc.tile_pool(name="ps_o01", bufs=2, space="PSUM") as ps_o01,
            tc.tile_pool(name="ps_o23", bufs=1, space="PSUM") as ps_o23,
            tc.tile_pool(name="ps_o4", bufs=1, space="PSUM") as ps_o4,
        ):
            ident = constp.tile([128, 128], f32)
            masks.make_identity(nc, ident[:])

            # warm the PE p-state while the first DMAs are in flight
            warm_ps = ps_sim.tile([PT, WS], f32, tag="sim_ps")
            for i in range(12):
                nc.tensor.transpose(warm_ps[:, (i % 2) * 128:(i % 2 + 1) * 128],
                                    ident[:], ident[:])
            # touch the ACT function tables off the critical path
            dummy = constp.tile([1, 8], f32)
            nc.scalar.sqrt(dummy[:, 0:2], ident[0:1, 0:2])
            nc.scalar.mul(dummy[:, 4:6], ident[0:1, 0:2], 1.0)

            # ---- all input loads, issued up front (prefetch) ------------
            # order: q chunk 0 (gates mm1 of tile 0), nkT (gates mm1),
            # q chunk 1, key halves (gate mm2 only), q chunk 2
            qtiles = {}
            nkT = keyp.tile([128, KC * WS], bf16, tag="nkT")
            key_sb = keyp.tile([128, W * D], bf16, tag="key_sb")

            def load_qchunk(ci):
                t0, t1 = QCHUNKS[ci]
                nq = (t1 - t0) * PT
                qc = keyp.tile([128, KC * nq], bf16, tag=f"qc{ci}",
                               name=f"qc{ci}")
                nc.sync.dma_start(
                    qc[:].rearrange("p (c q) -> p c q", q=nq),
                    qT_d[:, :, t0 * PT:t1 * PT].rearrange("c p q -> p c q"))
                for t in range(t0, t1):
                    qtiles[t] = (qc, t0, nq)

            load_qchunk(0)
            nc.sync.dma_start(
                nkT[:].rearrange("p (c n) -> p c n", n=WS),
                nkT_d[:, :, :].rearrange("c p n -> p c n"))
            load_qchunk(1)
            # key replicated into both partition halves so mm2's rhs base
            # partition matches simT's (w%2)*64 slice
            for hh in range(2):
                nc.sync.dma_start(
                    key_sb[hh * S:(hh + 1) * S, :],
                    ks_d[:, :, :].rearrange("s w d -> s (w d)"))
            load_qchunk(2)

            st = {}  # per-tile state for the 1-tile software pipeline

            def emit_back_half(u):
                simT, rinv = st.pop(u)
                # mm2 per w; w0/w1 share a 2-bank psum tile for the merged
                # DVE copy, w2/w3 share one for ACT's singles
                o01 = ps_o01.tile([PT, 1024], f32, tag="o01")
                o23 = ps_o23.tile([PT, 1024], f32, tag="o23")
                o4 = ps_o4.tile([PT, 512], f32, tag="o4")
                for w in range(W):
                    dst = (o01[:, (w % 2) * 512:(w % 2) * 512 + 512] if w < 2
                           else o23[:, (w % 2) * 512:(w % 2) * 512 + 512] if w < 4
                           else o4[:])
                    hh = w % 2
                    nc.tensor.matmul(
                        dst,
                        simT[hh * S:(hh + 1) * S, w // 2, :],
                        key_sb[hh * S:(hh + 1) * S, w * D:(w + 1) * D],
                        start=True, stop=True)

                if u % ST_B == 0:
                    st["ob"] = outp.tile([PT, ST_B * W * D], i8, tag="ob",
                                         name="ob")
                ob = st["ob"]
                ob_off = (u % ST_B) * W * D
                # w0/w1: one DVE tensor_tensor, rinv broadcast along d
                nc.vector.tensor_tensor(
                    ob[:, ob_off:ob_off + 1024].rearrange(
                        "p (w d) -> p w d", d=512),
                    o01[:].rearrange("p (w d) -> p w d", d=512),
                    rinv[:, 0:2, None].broadcast_to((PT, 2, 512)),
                    mybir.AluOpType.mult)
                # w2/w3/w4: ACT scaled copies (per-partition scale AP)
                nc.scalar.mul(ob[:, ob_off + 1024:ob_off + 1536],
                              o23[:, 0:512], rinv[:, 2:3])
                nc.scalar.mul(ob[:, ob_off + 1536:ob_off + 2048],
                              o23[:, 512:1024], rinv[:, 3:4])
                nc.scalar.mul(ob[:, ob_off + 2048:ob_off + 2560],
                              o4[:], rinv[:, 4:5])

                if u % ST_B == ST_B - 1:
                    g0 = u - ST_B + 1
                    nc.sync.dma_start(
                        o_d[g0 * PT:(g0 + ST_B) * PT, :]
                        .rearrange("(j p) d -> p j d", p=PT),
                        ob[:].rearrange("p (j d) -> p j d", d=W * D))

            # ---- main loop over query tiles -----------------------------
            for t in range(NT):
                qc, t0, nq = qtiles[t]
                qoff = (t - t0) * PT

                # mm1: sim[q, ws] = sum_D qT^T nkT
                sim_ps = ps_sim.tile([PT, WS], f32, tag="sim_ps")
                for c in range(KC):
                    nc.tensor.matmul(
                        sim_ps[:],
                        qc[:, c * nq + qoff:c * nq + qoff + PT],
                        nkT[:, c * WS:(c + 1) * WS],
                        start=(c == 0), stop=(c == KC - 1))

                # sim -> SBUF bf16 (feeds the xbar transpose + Pool square)
                simc = simcp.tile([PT, WPAD], bf16, tag="simc")
                nc.vector.tensor_copy(simc[:, 0:WS], sim_ps[:])
                if t < 4:  # zero each rotating buffer's xbar pad once
                    nc.gpsimd.memset(simc[:, WS:WPAD], 0)

                # norm path: ssq = sum_s sim^2, rinv = rsqrt(ssq)
                sq = sqp.tile([PT, WS], f32, tag="sq")
                nc.gpsimd.tensor_mul(sq[:], simc[:, 0:WS], simc[:, 0:WS])
                ssq = nrmp.tile([PT, 8], f32, tag="ssq")
                nc.vector.reduce_sum(
                    out=ssq[:, 0:W],
                    in_=sq[:].rearrange("p (w s) -> p w s", s=S),
                    axis=mybir.AxisListType.X)
                nrm = nrmp.tile([PT, 8], f32, tag="nrm")
                nc.scalar.sqrt(nrm[:, 0:W], ssq[:, 0:W])
                rinv = nrmp.tile([PT, 8], f32, tag="rinv")
                nc.vector.reciprocal(rinv[:, 0:W], nrm[:, 0:W])

                # xbar transpose: simT[s2w, wgrp, q] = simc[q, wgrp*128+s2w]
                simT = simtp.tile([128, 3, PT], bf16, tag="simT")
                nc.sync.dma_start_transpose(simT[:], simc[:])

                st[t] = (simT, rinv)
                if t >= SKEW:
                    emit_back_half(t - SKEW)
            for u in range(NT - SKEW, NT):
                emit_back_half(u)

    nc.compile()
    return nc


def _get_nc():
    if "nc" not in _CACHE:
        _CACHE["nc"] = _build()
    return _CACHE["nc"]


def kernel(query: np.ndarray, key: np.ndarray) -> np.ndarray:
    import ml_dtypes
    from concourse.bass_utils import run_bass_kernel_spmd

    bf16 = ml_dtypes.bfloat16
    query = np.asarray(query, dtype=np.float32)
    key = np.asarray(key, dtype=np.float32)
    assert query.shape == (B, Q, D) and key.shape == (B, W, S, D)

    nc = _get_nc()
    half = Q // 2
    in_maps = []
    scales = []
    for core in range(NCORES):
        b, h = divmod(core, 2)
        qs = query[b, h * half:(h + 1) * half, :]             # [QSH, D]
        kb = key[b].reshape(W * S, D).astype(np.float64)      # [WS, D]
        n = np.linalg.norm(kb, axis=-1, keepdims=True)
        nk = kb / np.maximum(n, 1e-12)
        # int8 bound: |out*rinv| <= max_col ||key[w,:,d]||_2
        s0 = MARGIN * np.sqrt((key[b].astype(np.float64) ** 2)
                              .sum(axis=1)).max()
        scales.append(np.float32(s0 / 127.0))
        ks = (key[b].transpose(1, 0, 2) * (127.0 / s0))       # [S, W, D]
        in_maps.append({
            "qT": np.ascontiguousarray(qs.T).reshape(KC, 128, QSH)
                    .astype(bf16),
            "nkT": np.ascontiguousarray(nk.T.astype(np.float32))
                    .reshape(KC, 128, WS).astype(bf16),
            "ks": np.ascontiguousarray(ks).astype(bf16),
        })
    res = run_bass_kernel_spmd(nc, in_maps, list(range(NCORES)))
    out = np.empty((B, Q, W, D), dtype=np.float32)
    for core in range(NCORES):
        b, h = divmod(core, 2)
        o8 = res.results[core]["out"].reshape(half, W, D)
        out[b, h * half:(h + 1) * half] = \
            o8.astype(np.float32) * scales[core]
    return out
